# revision 8
# baseline (speedup 1.0000x reference)
"""Trainium2 Bass kernel for nn_Net_73710228734901.

The network's post-gather graph (concat -> Conv3d -> spatial mean -> Linear)
is entirely linear in the gathered pixels, and the gathers / avg-pool /
1x1-conv are linear in the inputs.  Since the output is only [B, 1], the
whole model collapses to

    out[b] = lin_b + <W1, x1[b]> + <W2, x2[b]> + <W4, share[b]> + <W3, x3[b]>

with fixed per-element weight tensors W* computed (cheaply, on host) from
c_w / conv3d_w / lin_w / idx_h / idx_w.  The device kernel is then a pure
memory-bound weighted reduction over the big activations.

Traffic optimizations (per core, channel-sharded 8 ways):
  * x1/x2/share only contribute through their per-channel 7x7 crop
    window (49 of 196 positions; the folded weights are exactly zero
    elsewhere), so the host packs just those 49 values per channel.
  * x3's folded weights are dense (the 1x1 conv mixes all
    output-channel crops), so x3 streams in full.
  * fp16 activations and weights: 18.8 MB/core, ~45us at the
    16-DMA-engine cap (424 GB/s/core).

Both streams are fully buffered in SBUF (no DMA ever waits on compute,
avoiding head-of-line blocking on the single hardware DGE queue), and
compute is split across all four engines so it hides under the stream:
  * PE lane: 423 of the 1127 reduction columns are host-packed
    TRANSPOSED ([128-row chunk, 64 batches]); each chunk is one rank-1
    matmul psum[1,64] += w_c^T @ x_c (~90ns issue-bound).
  * Row lanes (remaining 704 columns, per-batch [128, 704] tiles):
    28 batches on DVE scalar_tensor_tensor (fused mult+reduce, 1x) and
    36 batches on DVE tensor_tensor (fp16 2x) + Scalar-engine
    activation(Copy) whose accum_out does the free-dim sum.
"""

import numpy as np

import concourse.bacc as bacc
import concourse.mybir as mybir
from concourse.bass_utils import run_bass_kernel_spmd
from concourse.tile import TileContext

NCORES = 8
NB = 64           # full batch, all on every core (channel sharding)
F1 = 49           # 7*7 cropped positions (x1/x2/share shards: 128 ch/core)
F3 = 980          # x3 shard: 160 ch * 784 pos / 128 partitions
F_TOT = 3 * F1 + F3   # 1127 reduction columns per partition
C_PE = 564        # columns routed to the PE (transposed) lane
F_ROW = F_TOT - C_PE  # 704 columns in the per-batch row lane
BLK = 8           # batches per row-stream DMA chunk
CBLK = 47         # chunks per PE-stream DMA chunk (9 blocks)
N_STT = 26        # of every 64 batches, this many take the fused DVE path
W_SCALE = 1024.0  # weights pre-scaled by 2^10 so fp16 products avoid
                  # subnormals; undone exactly in the final combine

_F32 = mybir.dt.float32
_F16 = mybir.dt.float16


def _build_fold(c_w, conv3d_w, lin_w, lin_b, idx_h, idx_w):
    """Collapse conv3d+mean+linear into per-element weights (float64 host math).

    Returns A: [1024, 14, 14] (quadrant weights in gathered coordinates)
    and Ws3: [1280, 784] float32 (dense weights on the raw x3 grid).
    """
    c_w = c_w.astype(np.float64)
    conv3d_w = conv3d_w.astype(np.float64)
    lin_w = lin_w.astype(np.float64)

    # W2[c = i*64+dd, kh, kw] = sum_{o,d,kd: 3d-4+kd=dd} lin_w[o*24+d] * conv3d_w[o,i,kd,kh,kw]
    W2 = np.zeros((1024, 3, 3), np.float64)
    o_idx = np.arange(32) * 24
    i_idx = np.arange(16) * 64
    for d in range(24):
        for kd in range(3):
            dd = 3 * d - 4 + kd
            if 0 <= dd < 64:
                W2[i_idx + dd] += np.einsum(
                    'o,oikl->ikl', lin_w[o_idx + d, 0], conv3d_w[:, :, kd])

    # Mean over the 14x14 conv output folds each (kh,kw) tap into a border mask.
    M = np.zeros((3, 3, 14, 14), np.float64)
    rng = {0: (0, 13), 1: (0, 14), 2: (1, 14)}
    for kh in range(3):
        for kw in range(3):
            r0, r1 = rng[kh]
            c0, c1 = rng[kw]
            M[kh, kw, r0:r1, c0:c1] = 1.0
    A = np.einsum('ckl,klrs->crs', W2, M) / 196.0   # [1024, 14, 14]

    # x3 path: scatter quadrant 3's 7x7 weights to the pooled grid at the
    # per-channel crop offset, pull back through the 1x1 conv ...
    Ws3c = np.zeros((1024, 14, 14), np.float64)
    ci = np.arange(1024)[:, None, None]
    ri = (idx_h[2][:, None] + np.arange(7))[:, :, None]
    wi = (idx_w[2][:, None] + np.arange(7))[:, None, :]
    Ws3c[ci, ri, wi] = A[:, 0:7, 7:14]
    Wpool = np.einsum('oc,ohw->chw', c_w, Ws3c)     # [1280, 14, 14]
    # ... and through avg_pool2d(5, stride 2, pad 2) (transposed scatter).
    Ws3 = np.zeros((1280, 28, 28), np.float64)
    for dh in range(-2, 3):
        for dw in range(-2, 3):
            hs = [h for h in range(14) if 0 <= 2 * h + dh < 28]
            ws = [w for w in range(14) if 0 <= 2 * w + dw < 28]
            H = [2 * h + dh for h in hs]
            W_ = [2 * w + dw for w in ws]
            Ws3[:, np.ix_(H, W_)[0], np.ix_(H, W_)[1]] += \
                Wpool[:, np.ix_(hs, ws)[0], np.ix_(hs, ws)[1]] / 25.0

    return A, Ws3.reshape(1280, 784).astype(np.float32)


def _crop(x, ih, iw):
    """Gather per-channel 7x7 windows: [B,1024,14,14] -> [B,1024,49]."""
    n = x.shape[1]
    ci = np.arange(n)[:, None, None]
    ri = (ih[:, None] + np.arange(7))[:, :, None]
    wi = (iw[:, None] + np.arange(7))[:, None, :]
    return x[:, ci, ri, wi].reshape(x.shape[0], n, 49)


def _build_bass(blk=BLK):
    nc = bacc.Bacc("TRN2")
    xrow = nc.dram_tensor("xrow", [128, NB, F_ROW], _F16, kind="ExternalInput")
    xtr = nc.dram_tensor("xtr", [128, C_PE, NB], _F16, kind="ExternalInput")
    # single fp16 weight tensor: row-lane cols 0..F_ROW, PE cols after
    wall = nc.dram_tensor("wall", [128, F_TOT], _F16, kind="ExternalInput")
    # out rows 0..127: per-partition row-lane partials; row 128: PE partial
    out = nc.dram_tensor("out", [129, NB], _F32, kind="ExternalOutput")

    n_rblk = NB // blk + 1          # last 8 batches go as two half-blocks
    n_cblk = C_PE // CBLK
    assert n_cblk * CBLK == C_PE

    with TileContext(nc) as tc:
        with (
            tc.tile_pool(name="cpool", bufs=1) as cpool,
            tc.tile_pool(name="xpool", bufs=n_rblk) as xpool,
            tc.tile_pool(name="tpool", bufs=n_cblk) as tpool,
            tc.tile_pool(name="gpool", bufs=3) as gpool,
            tc.tile_pool(name="zpool", bufs=2) as zpool,
            tc.tile_pool(name="apool", bufs=1) as apool,
            tc.tile_pool(name="ppool", bufs=1, space="PSUM") as ppool,
        ):
            wt = cpool.tile([128, F_TOT], _F16)
            nc.scalar.dma_start(out=wt[:], in_=wall[:, :])

            acc = apool.tile([128, NB], _F32)
            ps = ppool.tile([1, NB], _F32)

            # Interleave row-stream and PE-stream blocks, with the PE
            # stream skewed ~1.4x ahead so the Tensor engine finishes
            # well before the end (its semaphore cleanup then overlaps
            # the remaining row stream instead of trailing the kernel).
            row_i, chunk_i = 0, 0
            prog = []
            while row_i < n_rblk or chunk_i < n_cblk:
                if chunk_i * n_rblk * 10 <= row_i * n_cblk * 14 \
                        and chunk_i < n_cblk:
                    prog.append(('c', chunk_i)); chunk_i += 1
                else:
                    prog.append(('r', row_i)); row_i += 1

            pe_c = 0
            for kind, idx in prog:
                if kind == 'c':
                    ct = tpool.tile([128, CBLK, NB], _F16, tag="ct")
                    nc.sync.dma_start(
                        out=ct[:], in_=xtr[:, idx * CBLK:(idx + 1) * CBLK, :])
                    for q in range(CBLK):
                        c = idx * CBLK + q
                        nc.tensor.matmul(
                            ps[:], lhsT=wt[:, F_ROW + c:F_ROW + c + 1],
                            rhs=ct[:, q, :],
                            start=(pe_c == 0), stop=(pe_c == C_PE - 1))
                        pe_c += 1
                else:
                    if idx < n_rblk - 2:
                        b0, nbat, tag = idx * blk, blk, "xt"
                    else:
                        half = blk // 2
                        b0 = (n_rblk - 2) * blk + (idx - n_rblk + 2) * half
                        nbat, tag = half, "xth"
                    xt = xpool.tile([128, nbat, F_ROW], _F16, tag=tag)
                    nc.sync.dma_start(
                        out=xt[:], in_=xrow[:, b0:b0 + nbat, :])
                    # In the tail blocks emit the ACT-lane batches first
                    # so the Scalar engine finishes before DVE.
                    order = list(range(nbat))
                    if idx >= n_rblk - 2:
                        order.sort(key=lambda j: (b0 + j) * N_STT % NB < N_STT)
                    for j in order:
                        b = b0 + j
                        if (b * N_STT) % NB < N_STT:
                            scr = zpool.tile([128, F_ROW], _F16, tag="scr")
                            # Fused multiply + free-dim sum in one DVE pass.
                            nc.vector.scalar_tensor_tensor(
                                out=scr[:],
                                in0=xt[:, j, :],
                                scalar=1.0,
                                in1=wt[:, 0:F_ROW],
                                op0=mybir.AluOpType.mult,
                                op1=mybir.AluOpType.mult,
                                accum_out=acc[:, b:b + 1],
                            )
                        else:
                            prod = gpool.tile([128, F_ROW], _F16, tag="prod")
                            nc.vector.tensor_tensor(
                                prod[:], xt[:, j, :], wt[:, 0:F_ROW],
                                mybir.AluOpType.mult)
                            sink = zpool.tile([128, F_ROW], _F16, tag="sink")
                            nc.scalar.activation(
                                sink[:], prod[:],
                                mybir.ActivationFunctionType.Copy,
                                accum_out=acc[:, b:b + 1])

            # Ship the raw partials; the host does the tiny partition
            # sum, un-scaling, and lin_b add.  (Keeping the Tensor
            # engine out of the tail lets its cleanup overlap the
            # stream.)
            res = apool.tile([1, NB], _F32)
            nc.vector.tensor_copy(res[:], ps[:])
            nc.sync.dma_start(out=out[0:128, :], in_=acc[:])
            nc.sync.dma_start(out=out[128:129, :], in_=res[:])
    nc.finalize()
    return nc


def _shard_inputs(x1, x2, x3, share_feature, A, Ws3, lin_b, idx_h, idx_w):
    # Crop activations and weights to the 7x7 gather windows.
    x1c = _crop(x1.reshape(NB, 1024, 14, 14), idx_h[0], idx_w[0])
    x2c = _crop(x2.reshape(NB, 1024, 14, 14), idx_h[1], idx_w[1])
    shc = _crop(share_feature.reshape(NB, 1024, 14, 14), idx_h[3], idx_w[3])
    Wc1 = A[:, 0:7, 0:7].reshape(1024, 49).astype(np.float32)
    Wc2 = A[:, 7:14, 0:7].reshape(1024, 49).astype(np.float32)
    Wc4 = A[:, 7:14, 7:14].reshape(1024, 49).astype(np.float32)

    in_maps = []
    for m in range(NCORES):
        cs = slice(m * 128, (m + 1) * 128)
        cs3 = slice(m * 160, (m + 1) * 160)
        xin = np.concatenate([
            x1c[:, cs],
            x2c[:, cs],
            shc[:, cs],
            x3[:, cs3].reshape(NB, 128, F3),
        ], axis=2)                                   # [64, 128, 1127]
        wfull = np.concatenate([
            Wc1[cs],
            Wc2[cs],
            Wc4[cs],
            Ws3[cs3].reshape(128, F3),
        ], axis=1) * W_SCALE                         # [128, 1127]
        # First F_ROW columns stream row-major; the last C_PE columns
        # stream transposed for the PE lane.
        xr = np.ascontiguousarray(
            xin[:, :, :F_ROW].transpose(1, 0, 2), dtype=np.float16)
        xt = np.ascontiguousarray(
            xin[:, :, F_ROW:].transpose(1, 2, 0), dtype=np.float16)
        in_maps.append({'xrow': xr, 'xtr': xt,
                        'wall': np.ascontiguousarray(wfull, np.float16)})
    return in_maps


def _ensure_ntff_hook():
    """Make `trace=True` (e.g. BASS_TRACE=1) work under axon even when the
    image's antenv package lacks axon_hooks: register an equivalent module
    backed by the ctypes NTFF hook from trn_agent_boot."""
    import sys
    import types
    try:
        import antenv.axon_hooks  # noqa: F401
        return
    except Exception:
        pass
    try:
        from trn_agent_boot import trn_boot
        hook = trn_boot._ntff_profile_via_ctypes('/opt/axon/libaxon_pjrt.so')
        mod = types.ModuleType('antenv.axon_hooks')
        mod.get_axon_ntff_profile_hook = lambda: hook
        mod.set_axon_ntff_profile_hook = lambda h: None
        sys.modules['antenv.axon_hooks'] = mod
    except Exception:
        pass


def _prepare(x1, x2, x3, share_feature, c_w, conv3d_w, lin_w, lin_b,
             idx_h, idx_w):
    A, Ws3 = _build_fold(c_w, conv3d_w, lin_w, lin_b, idx_h, idx_w)
    in_maps = _shard_inputs(x1, x2, x3, share_feature, A, Ws3, lin_b,
                            idx_h, idx_w)
    nc = _build_bass()
    return in_maps, nc


def kernel(x1, x2, x3, share_feature, c_w, conv3d_w, lin_w, lin_b,
           idx_h, idx_w):
    x1, x2, x3 = np.asarray(x1), np.asarray(x2), np.asarray(x3)
    share_feature = np.asarray(share_feature)
    c_w, conv3d_w = np.asarray(c_w), np.asarray(conv3d_w)
    lin_w, lin_b = np.asarray(lin_w), np.asarray(lin_b)
    idx_h, idx_w = np.asarray(idx_h), np.asarray(idx_w)
    _ensure_ntff_hook()
    in_maps, nc = _prepare(x1, x2, x3, share_feature, c_w, conv3d_w,
                           lin_w, lin_b, idx_h, idx_w)
    res = run_bass_kernel_spmd(nc, in_maps, core_ids=list(range(NCORES)))
    parts = np.stack([np.asarray(r['out'], np.float64).sum(axis=0)
                      for r in res.results])                  # [8, 64]
    return (parts.sum(axis=0) / W_SCALE + float(lin_b[0])) \
        .astype(np.float32).reshape(NB, 1)


# revision 9
# speedup vs baseline: 1.0984x; 1.0984x over previous
"""Trainium2 Bass kernel for nn_Net_73710228734901.

The network's post-gather graph (concat -> Conv3d -> spatial mean -> Linear)
is entirely linear in the gathered pixels, and the gathers / avg-pool /
1x1-conv are linear in the inputs.  Since the output is only [B, 1], the
whole model collapses to

    out[b] = lin_b + <W1, x1[b]> + <W2, x2[b]> + <W4, share[b]> + <W3, x3[b]>

with fixed per-element weight tensors W* computed (cheaply, on host) from
c_w / conv3d_w / lin_w / idx_h / idx_w.  The device kernel is then a pure
memory-bound weighted reduction over the big activations.

Traffic optimizations (per core, channel-sharded 8 ways):
  * x1/x2/share only contribute through their per-channel 7x7 crop
    window (49 of 196 positions; the folded weights are exactly zero
    elsewhere), so the host packs just those 49 values per channel.
  * x3's folded weights are dense (the 1x1 conv mixes all
    output-channel crops), so x3 streams in full.
  * fp16 activations and weights: 18.8 MB/core, ~45us at the
    16-DMA-engine cap (424 GB/s/core).

Both streams are fully buffered in SBUF (no DMA ever waits on compute,
avoiding head-of-line blocking on the single hardware DGE queue), and
compute is split across all four engines so it hides under the stream:
  * PE lane: 423 of the 1127 reduction columns are host-packed
    TRANSPOSED ([128-row chunk, 64 batches]); each chunk is one rank-1
    matmul psum[1,64] += w_c^T @ x_c (~90ns issue-bound).
  * Row lanes (remaining 704 columns, per-batch [128, 704] tiles):
    28 batches on DVE scalar_tensor_tensor (fused mult+reduce, 1x) and
    36 batches on DVE tensor_tensor (fp16 2x) + Scalar-engine
    activation(Copy) whose accum_out does the free-dim sum.
"""

import numpy as np

import concourse.bacc as bacc
import concourse.mybir as mybir
from concourse.bass_utils import run_bass_kernel_spmd
from concourse.tile import TileContext

NCORES = 8
NB = 64           # full batch, all on every core (channel sharding)
F1 = 49           # 7*7 cropped positions (x1/x2/share shards: 128 ch/core)
F3 = 980          # x3 shard: 160 ch * 784 pos / 128 partitions
F_TOT = 3 * F1 + F3   # 1127 reduction columns per partition
C_PE = 517        # columns routed to the PE (transposed) lane
F_ROW = F_TOT - C_PE  # 704 columns in the per-batch row lane
BLK = 8           # batches per row-stream DMA chunk
CBLK = 47         # chunks per PE-stream DMA chunk (9 blocks)
N_STT = 30        # of every 64 batches, this many take the fused DVE path
W_SCALE = 1024.0  # weights pre-scaled by 2^10 so fp16 products avoid
                  # subnormals; undone exactly in the final combine

_F32 = mybir.dt.float32
_F16 = mybir.dt.float16


def _build_fold(c_w, conv3d_w, lin_w, lin_b, idx_h, idx_w):
    """Collapse conv3d+mean+linear into per-element weights (float64 host math).

    Returns A: [1024, 14, 14] (quadrant weights in gathered coordinates)
    and Ws3: [1280, 784] float32 (dense weights on the raw x3 grid).
    """
    c_w = c_w.astype(np.float64)
    conv3d_w = conv3d_w.astype(np.float64)
    lin_w = lin_w.astype(np.float64)

    # W2[c = i*64+dd, kh, kw] = sum_{o,d,kd: 3d-4+kd=dd} lin_w[o*24+d] * conv3d_w[o,i,kd,kh,kw]
    W2 = np.zeros((1024, 3, 3), np.float64)
    o_idx = np.arange(32) * 24
    i_idx = np.arange(16) * 64
    for d in range(24):
        for kd in range(3):
            dd = 3 * d - 4 + kd
            if 0 <= dd < 64:
                W2[i_idx + dd] += np.einsum(
                    'o,oikl->ikl', lin_w[o_idx + d, 0], conv3d_w[:, :, kd])

    # Mean over the 14x14 conv output folds each (kh,kw) tap into a border mask.
    M = np.zeros((3, 3, 14, 14), np.float64)
    rng = {0: (0, 13), 1: (0, 14), 2: (1, 14)}
    for kh in range(3):
        for kw in range(3):
            r0, r1 = rng[kh]
            c0, c1 = rng[kw]
            M[kh, kw, r0:r1, c0:c1] = 1.0
    A = np.einsum('ckl,klrs->crs', W2, M) / 196.0   # [1024, 14, 14]

    # x3 path: scatter quadrant 3's 7x7 weights to the pooled grid at the
    # per-channel crop offset, pull back through the 1x1 conv ...
    Ws3c = np.zeros((1024, 14, 14), np.float64)
    ci = np.arange(1024)[:, None, None]
    ri = (idx_h[2][:, None] + np.arange(7))[:, :, None]
    wi = (idx_w[2][:, None] + np.arange(7))[:, None, :]
    Ws3c[ci, ri, wi] = A[:, 0:7, 7:14]
    Wpool = np.einsum('oc,ohw->chw', c_w, Ws3c)     # [1280, 14, 14]
    # ... and through avg_pool2d(5, stride 2, pad 2) (transposed scatter).
    Ws3 = np.zeros((1280, 28, 28), np.float64)
    for dh in range(-2, 3):
        for dw in range(-2, 3):
            hs = [h for h in range(14) if 0 <= 2 * h + dh < 28]
            ws = [w for w in range(14) if 0 <= 2 * w + dw < 28]
            H = [2 * h + dh for h in hs]
            W_ = [2 * w + dw for w in ws]
            Ws3[:, np.ix_(H, W_)[0], np.ix_(H, W_)[1]] += \
                Wpool[:, np.ix_(hs, ws)[0], np.ix_(hs, ws)[1]] / 25.0

    return A, Ws3.reshape(1280, 784).astype(np.float32)


def _crop(x, ih, iw):
    """Gather per-channel 7x7 windows: [B,1024,14,14] -> [B,1024,49]."""
    n = x.shape[1]
    ci = np.arange(n)[:, None, None]
    ri = (ih[:, None] + np.arange(7))[:, :, None]
    wi = (iw[:, None] + np.arange(7))[:, None, :]
    return x[:, ci, ri, wi].reshape(x.shape[0], n, 49)


def _build_bass(blk=BLK):
    nc = bacc.Bacc("TRN2")
    xrow = nc.dram_tensor("xrow", [128, NB, F_ROW], _F16, kind="ExternalInput")
    xtr = nc.dram_tensor("xtr", [128, C_PE, NB], _F16, kind="ExternalInput")
    # single fp16 weight tensor: row-lane cols 0..F_ROW, PE cols after
    wall = nc.dram_tensor("wall", [128, F_TOT], _F16, kind="ExternalInput")
    # out rows 0..127: per-partition row-lane partials; row 128: PE partial
    out = nc.dram_tensor("out", [129, NB], _F32, kind="ExternalOutput")

    n_rblk = NB // blk + 1          # last 8 batches go as two half-blocks
    n_cblk = C_PE // CBLK
    assert n_cblk * CBLK == C_PE

    with TileContext(nc) as tc:
        with (
            tc.tile_pool(name="cpool", bufs=1) as cpool,
            tc.tile_pool(name="xpool", bufs=n_rblk) as xpool,
            tc.tile_pool(name="tpool", bufs=n_cblk) as tpool,
            tc.tile_pool(name="gpool", bufs=3) as gpool,
            tc.tile_pool(name="zpool", bufs=2) as zpool,
            tc.tile_pool(name="apool", bufs=1) as apool,
            tc.tile_pool(name="ppool", bufs=1, space="PSUM") as ppool,
        ):
            wt = cpool.tile([128, F_TOT], _F16)
            nc.sync.dma_start(out=wt[:], in_=wall[:, :])

            acc = apool.tile([128, NB], _F32)
            ps = ppool.tile([1, NB], _F32)

            # Interleave row-stream and PE-stream blocks, with the PE
            # stream skewed ~1.4x ahead so the Tensor engine finishes
            # well before the end (its semaphore cleanup then overlaps
            # the remaining row stream instead of trailing the kernel).
            row_i, chunk_i = 0, 0
            prog = []
            while row_i < n_rblk or chunk_i < n_cblk:
                if chunk_i * n_rblk <= row_i * n_cblk and chunk_i < n_cblk:
                    prog.append(('c', chunk_i)); chunk_i += 1
                else:
                    prog.append(('r', row_i)); row_i += 1

            pe_c = 0
            for kind, idx in prog:
                if kind == 'c':
                    ct = tpool.tile([128, CBLK, NB], _F16, tag="ct")
                    nc.scalar.dma_start(
                        out=ct[:], in_=xtr[:, idx * CBLK:(idx + 1) * CBLK, :])
                    for q in range(CBLK):
                        c = idx * CBLK + q
                        nc.tensor.matmul(
                            ps[:], lhsT=wt[:, F_ROW + c:F_ROW + c + 1],
                            rhs=ct[:, q, :],
                            start=(pe_c == 0), stop=(pe_c == C_PE - 1))
                        pe_c += 1
                else:
                    if idx < n_rblk - 2:
                        b0, nbat, tag = idx * blk, blk, "xt"
                    else:
                        half = blk // 2
                        b0 = (n_rblk - 2) * blk + (idx - n_rblk + 2) * half
                        nbat, tag = half, "xth"
                    xt = xpool.tile([128, nbat, F_ROW], _F16, tag=tag)
                    nc.sync.dma_start(
                        out=xt[:], in_=xrow[:, b0:b0 + nbat, :])
                    # In the tail blocks emit the ACT-lane batches first
                    # so the Scalar engine finishes before DVE.
                    order = list(range(nbat))
                    if idx >= n_rblk - 2:
                        order.sort(key=lambda j: (b0 + j) * N_STT % NB < N_STT)
                    for j in order:
                        b = b0 + j
                        if (b * N_STT) % NB < N_STT:
                            scr = zpool.tile([128, F_ROW], _F16, tag="scr")
                            # Fused multiply + free-dim sum in one DVE pass.
                            nc.vector.scalar_tensor_tensor(
                                out=scr[:],
                                in0=xt[:, j, :],
                                scalar=1.0,
                                in1=wt[:, 0:F_ROW],
                                op0=mybir.AluOpType.mult,
                                op1=mybir.AluOpType.mult,
                                accum_out=acc[:, b:b + 1],
                            )
                        else:
                            prod = gpool.tile([128, F_ROW], _F16, tag="prod")
                            nc.vector.tensor_tensor(
                                prod[:], xt[:, j, :], wt[:, 0:F_ROW],
                                mybir.AluOpType.mult)
                            sink = zpool.tile([128, F_ROW], _F16, tag="sink")
                            nc.scalar.activation(
                                sink[:], prod[:],
                                mybir.ActivationFunctionType.Copy,
                                accum_out=acc[:, b:b + 1])

            # Ship the raw partials; the host does the tiny partition
            # sum, un-scaling, and lin_b add.  (Keeping the Tensor
            # engine out of the tail lets its cleanup overlap the
            # stream.)
            res = apool.tile([1, NB], _F32)
            nc.vector.tensor_copy(res[:], ps[:])
            nc.sync.dma_start(out=out[0:128, :], in_=acc[:])
            nc.sync.dma_start(out=out[128:129, :], in_=res[:])
    nc.finalize()
    return nc


def _shard_inputs(x1, x2, x3, share_feature, A, Ws3, lin_b, idx_h, idx_w):
    # Crop activations and weights to the 7x7 gather windows.
    x1c = _crop(x1.reshape(NB, 1024, 14, 14), idx_h[0], idx_w[0])
    x2c = _crop(x2.reshape(NB, 1024, 14, 14), idx_h[1], idx_w[1])
    shc = _crop(share_feature.reshape(NB, 1024, 14, 14), idx_h[3], idx_w[3])
    Wc1 = A[:, 0:7, 0:7].reshape(1024, 49).astype(np.float32)
    Wc2 = A[:, 7:14, 0:7].reshape(1024, 49).astype(np.float32)
    Wc4 = A[:, 7:14, 7:14].reshape(1024, 49).astype(np.float32)

    in_maps = []
    for m in range(NCORES):
        cs = slice(m * 128, (m + 1) * 128)
        cs3 = slice(m * 160, (m + 1) * 160)
        xin = np.concatenate([
            x1c[:, cs],
            x2c[:, cs],
            shc[:, cs],
            x3[:, cs3].reshape(NB, 128, F3),
        ], axis=2)                                   # [64, 128, 1127]
        wfull = np.concatenate([
            Wc1[cs],
            Wc2[cs],
            Wc4[cs],
            Ws3[cs3].reshape(128, F3),
        ], axis=1) * W_SCALE                         # [128, 1127]
        # First F_ROW columns stream row-major; the last C_PE columns
        # stream transposed for the PE lane.
        xr = np.ascontiguousarray(
            xin[:, :, :F_ROW].transpose(1, 0, 2), dtype=np.float16)
        xt = np.ascontiguousarray(
            xin[:, :, F_ROW:].transpose(1, 2, 0), dtype=np.float16)
        in_maps.append({'xrow': xr, 'xtr': xt,
                        'wall': np.ascontiguousarray(wfull, np.float16)})
    return in_maps


def _ensure_ntff_hook():
    """Make `trace=True` (e.g. BASS_TRACE=1) work under axon even when the
    image's antenv package lacks axon_hooks: register an equivalent module
    backed by the ctypes NTFF hook from trn_agent_boot."""
    import sys
    import types
    try:
        import antenv.axon_hooks  # noqa: F401
        return
    except Exception:
        pass
    try:
        from trn_agent_boot import trn_boot
        hook = trn_boot._ntff_profile_via_ctypes('/opt/axon/libaxon_pjrt.so')
        mod = types.ModuleType('antenv.axon_hooks')
        mod.get_axon_ntff_profile_hook = lambda: hook
        mod.set_axon_ntff_profile_hook = lambda h: None
        sys.modules['antenv.axon_hooks'] = mod
    except Exception:
        pass


def _prepare(x1, x2, x3, share_feature, c_w, conv3d_w, lin_w, lin_b,
             idx_h, idx_w):
    A, Ws3 = _build_fold(c_w, conv3d_w, lin_w, lin_b, idx_h, idx_w)
    in_maps = _shard_inputs(x1, x2, x3, share_feature, A, Ws3, lin_b,
                            idx_h, idx_w)
    nc = _build_bass()
    return in_maps, nc


def kernel(x1, x2, x3, share_feature, c_w, conv3d_w, lin_w, lin_b,
           idx_h, idx_w):
    x1, x2, x3 = np.asarray(x1), np.asarray(x2), np.asarray(x3)
    share_feature = np.asarray(share_feature)
    c_w, conv3d_w = np.asarray(c_w), np.asarray(conv3d_w)
    lin_w, lin_b = np.asarray(lin_w), np.asarray(lin_b)
    idx_h, idx_w = np.asarray(idx_h), np.asarray(idx_w)
    _ensure_ntff_hook()
    in_maps, nc = _prepare(x1, x2, x3, share_feature, c_w, conv3d_w,
                           lin_w, lin_b, idx_h, idx_w)
    res = run_bass_kernel_spmd(nc, in_maps, core_ids=list(range(NCORES)))
    parts = np.stack([np.asarray(r['out'], np.float64).sum(axis=0)
                      for r in res.results])                  # [8, 64]
    return (parts.sum(axis=0) / W_SCALE + float(lin_b[0])) \
        .astype(np.float32).reshape(NB, 1)


# revision 10
# speedup vs baseline: 1.1176x; 1.0175x over previous
"""Trainium2 Bass kernel for nn_Net_73710228734901.

The network's post-gather graph (concat -> Conv3d -> spatial mean -> Linear)
is entirely linear in the gathered pixels, and the gathers / avg-pool /
1x1-conv are linear in the inputs.  Since the output is only [B, 1], the
whole model collapses to

    out[b] = lin_b + <W1, x1[b]> + <W2, x2[b]> + <W4, share[b]> + <W3, x3[b]>

with fixed per-element weight tensors W* computed (cheaply, on host) from
c_w / conv3d_w / lin_w / idx_h / idx_w.  The device kernel is then a pure
memory-bound weighted reduction over the big activations.

Traffic optimizations (per core, channel-sharded 8 ways):
  * x1/x2/share only contribute through their per-channel 7x7 crop
    window (49 of 196 positions; the folded weights are exactly zero
    elsewhere), so the host packs just those 49 values per channel.
  * x3's folded weights are dense (the 1x1 conv mixes all
    output-channel crops), so x3 streams in full.
  * fp16 activations and weights: 18.8 MB/core, ~45us at the
    16-DMA-engine cap (424 GB/s/core).

Both streams are fully buffered in SBUF (no DMA ever waits on compute,
avoiding head-of-line blocking on the single hardware DGE queue), and
compute is split across all four engines so it hides under the stream:
  * PE lane: 423 of the 1127 reduction columns are host-packed
    TRANSPOSED ([128-row chunk, 64 batches]); each chunk is one rank-1
    matmul psum[1,64] += w_c^T @ x_c (~90ns issue-bound).
  * Row lanes (remaining 704 columns, per-batch [128, 704] tiles):
    28 batches on DVE scalar_tensor_tensor (fused mult+reduce, 1x) and
    36 batches on DVE tensor_tensor (fp16 2x) + Scalar-engine
    activation(Copy) whose accum_out does the free-dim sum.
"""

import numpy as np

import concourse.bacc as bacc
import concourse.mybir as mybir
from concourse.bass_utils import run_bass_kernel_spmd
from concourse.tile import TileContext

NCORES = 8
NB = 64           # full batch, all on every core (channel sharding)
F1 = 49           # 7*7 cropped positions (x1/x2/share shards: 128 ch/core)
F3 = 980          # x3 shard: 160 ch * 784 pos / 128 partitions
F_TOT = 3 * F1 + F3   # 1127 reduction columns per partition
C_PE = 517        # columns routed to the PE (transposed) lane
F_ROW = F_TOT - C_PE  # 704 columns in the per-batch row lane
BLK = 8           # batches per row-stream DMA chunk
CBLK = 47         # chunks per PE-stream DMA chunk (9 blocks)
N_STT = 30        # of every 64 batches, this many take the fused DVE path
W_SCALE = 1024.0  # weights pre-scaled by 2^10 so fp16 products avoid
                  # subnormals; undone exactly in the final combine

_F32 = mybir.dt.float32
_F16 = mybir.dt.float16


def _build_fold(c_w, conv3d_w, lin_w, lin_b, idx_h, idx_w):
    """Collapse conv3d+mean+linear into per-element weights (float64 host math).

    Returns A: [1024, 14, 14] (quadrant weights in gathered coordinates)
    and Ws3: [1280, 784] float32 (dense weights on the raw x3 grid).
    """
    c_w = c_w.astype(np.float64)
    conv3d_w = conv3d_w.astype(np.float64)
    lin_w = lin_w.astype(np.float64)

    # W2[c = i*64+dd, kh, kw] = sum_{o,d,kd: 3d-4+kd=dd} lin_w[o*24+d] * conv3d_w[o,i,kd,kh,kw]
    W2 = np.zeros((1024, 3, 3), np.float64)
    o_idx = np.arange(32) * 24
    i_idx = np.arange(16) * 64
    for d in range(24):
        for kd in range(3):
            dd = 3 * d - 4 + kd
            if 0 <= dd < 64:
                W2[i_idx + dd] += np.einsum(
                    'o,oikl->ikl', lin_w[o_idx + d, 0], conv3d_w[:, :, kd])

    # Mean over the 14x14 conv output folds each (kh,kw) tap into a border mask.
    M = np.zeros((3, 3, 14, 14), np.float64)
    rng = {0: (0, 13), 1: (0, 14), 2: (1, 14)}
    for kh in range(3):
        for kw in range(3):
            r0, r1 = rng[kh]
            c0, c1 = rng[kw]
            M[kh, kw, r0:r1, c0:c1] = 1.0
    A = np.einsum('ckl,klrs->crs', W2, M) / 196.0   # [1024, 14, 14]

    # x3 path: scatter quadrant 3's 7x7 weights to the pooled grid at the
    # per-channel crop offset, pull back through the 1x1 conv ...
    Ws3c = np.zeros((1024, 14, 14), np.float64)
    ci = np.arange(1024)[:, None, None]
    ri = (idx_h[2][:, None] + np.arange(7))[:, :, None]
    wi = (idx_w[2][:, None] + np.arange(7))[:, None, :]
    Ws3c[ci, ri, wi] = A[:, 0:7, 7:14]
    Wpool = np.einsum('oc,ohw->chw', c_w, Ws3c)     # [1280, 14, 14]
    # ... and through avg_pool2d(5, stride 2, pad 2) (transposed scatter).
    Ws3 = np.zeros((1280, 28, 28), np.float64)
    for dh in range(-2, 3):
        for dw in range(-2, 3):
            hs = [h for h in range(14) if 0 <= 2 * h + dh < 28]
            ws = [w for w in range(14) if 0 <= 2 * w + dw < 28]
            H = [2 * h + dh for h in hs]
            W_ = [2 * w + dw for w in ws]
            Ws3[:, np.ix_(H, W_)[0], np.ix_(H, W_)[1]] += \
                Wpool[:, np.ix_(hs, ws)[0], np.ix_(hs, ws)[1]] / 25.0

    return A, Ws3.reshape(1280, 784).astype(np.float32)


def _crop(x, ih, iw):
    """Gather per-channel 7x7 windows: [B,1024,14,14] -> [B,1024,49]."""
    n = x.shape[1]
    ci = np.arange(n)[:, None, None]
    ri = (ih[:, None] + np.arange(7))[:, :, None]
    wi = (iw[:, None] + np.arange(7))[:, None, :]
    return x[:, ci, ri, wi].reshape(x.shape[0], n, 49)


def _build_bass(blk=BLK):
    nc = bacc.Bacc("TRN2")
    xrow = nc.dram_tensor("xrow", [128, NB, F_ROW], _F16, kind="ExternalInput")
    xtr = nc.dram_tensor("xtr", [128, C_PE, NB], _F16, kind="ExternalInput")
    # single fp16 weight tensor: row-lane cols 0..F_ROW, PE cols after
    wall = nc.dram_tensor("wall", [128, F_TOT], _F16, kind="ExternalInput")
    # out rows 0..127: per-partition row-lane partials; row 128: PE partial
    out = nc.dram_tensor("out", [129, NB], _F32, kind="ExternalOutput")

    n_rblk = NB // blk + 1          # last 8 batches go as two half-blocks
    n_cblk = C_PE // CBLK
    assert n_cblk * CBLK == C_PE

    with TileContext(nc) as tc:
        with (
            tc.tile_pool(name="cpool", bufs=1) as cpool,
            tc.tile_pool(name="xpool", bufs=n_rblk) as xpool,
            tc.tile_pool(name="tpool", bufs=n_cblk) as tpool,
            tc.tile_pool(name="gpool", bufs=3) as gpool,
            tc.tile_pool(name="zpool", bufs=2) as zpool,
            tc.tile_pool(name="apool", bufs=1) as apool,
            tc.tile_pool(name="ppool", bufs=1, space="PSUM") as ppool,
        ):
            wt = cpool.tile([128, F_TOT], _F16)
            nc.sync.dma_start(out=wt[:], in_=wall[:, :])

            acc = apool.tile([128, NB], _F32)
            ps = ppool.tile([1, NB], _F32)

            # Interleave row-stream and PE-stream blocks, with the PE
            # stream skewed ~1.4x ahead so the Tensor engine finishes
            # well before the end (its semaphore cleanup then overlaps
            # the remaining row stream instead of trailing the kernel).
            row_i, chunk_i = 0, 0
            prog = []
            while row_i < n_rblk or chunk_i < n_cblk:
                if chunk_i * n_rblk <= row_i * n_cblk and chunk_i < n_cblk:
                    prog.append(('c', chunk_i)); chunk_i += 1
                else:
                    prog.append(('r', row_i)); row_i += 1

            pe_c = 0
            for kind, idx in prog:
                if kind == 'c':
                    ct = tpool.tile([128, CBLK, NB], _F16, tag="ct")
                    nc.sync.dma_start(
                        out=ct[:], in_=xtr[:, idx * CBLK:(idx + 1) * CBLK, :])
                    for q in range(CBLK):
                        c = idx * CBLK + q
                        nc.tensor.matmul(
                            ps[:], lhsT=wt[:, F_ROW + c:F_ROW + c + 1],
                            rhs=ct[:, q, :],
                            start=(pe_c == 0), stop=(pe_c == C_PE - 1))
                        pe_c += 1
                else:
                    if idx < n_rblk - 2:
                        b0, nbat, tag = idx * blk, blk, "xt"
                    else:
                        half = blk // 2
                        b0 = (n_rblk - 2) * blk + (idx - n_rblk + 2) * half
                        nbat, tag = half, "xth"
                    xt = xpool.tile([128, nbat, F_ROW], _F16, tag=tag)
                    nc.sync.dma_start(
                        out=xt[:], in_=xrow[:, b0:b0 + nbat, :])
                    # In the tail blocks emit the ACT-lane batches first
                    # so the Scalar engine finishes before DVE.
                    order = list(range(nbat))
                    if idx >= n_rblk - 2:
                        order.sort(key=lambda j: (b0 + j) * N_STT % NB < N_STT)
                    for j in order:
                        b = b0 + j
                        if (b * N_STT) % NB < N_STT:
                            scr = zpool.tile([128, F_ROW], _F16, tag="scr")
                            # Fused multiply + free-dim sum in one DVE pass.
                            nc.vector.scalar_tensor_tensor(
                                out=scr[:],
                                in0=xt[:, j, :],
                                scalar=1.0,
                                in1=wt[:, 0:F_ROW],
                                op0=mybir.AluOpType.mult,
                                op1=mybir.AluOpType.mult,
                                accum_out=acc[:, b:b + 1],
                            )
                        else:
                            prod = gpool.tile([128, F_ROW], _F16, tag="prod")
                            nc.vector.tensor_tensor(
                                prod[:], xt[:, j, :], wt[:, 0:F_ROW],
                                mybir.AluOpType.mult)
                            sink = zpool.tile([128, F_ROW], _F16, tag="sink")
                            nc.scalar.activation(
                                sink[:], prod[:],
                                mybir.ActivationFunctionType.Copy,
                                accum_out=acc[:, b:b + 1])

            # Ship the raw partials; the host does the tiny partition
            # sum, un-scaling, and lin_b add.  (Keeping the Tensor
            # engine out of the tail lets its cleanup overlap the
            # stream.)
            res = apool.tile([1, NB], _F32)
            nc.vector.tensor_copy(res[:], ps[:])
            nc.sync.dma_start(out=out[0:128, :], in_=acc[:])
            nc.sync.dma_start(out=out[128:129, :], in_=res[:])
    nc.finalize()
    return nc


def _shard_inputs(x1, x2, x3, share_feature, A, Ws3, lin_b, idx_h, idx_w):
    # Crop activations and weights to the 7x7 gather windows.
    x1c = _crop(x1.reshape(NB, 1024, 14, 14), idx_h[0], idx_w[0])
    x2c = _crop(x2.reshape(NB, 1024, 14, 14), idx_h[1], idx_w[1])
    shc = _crop(share_feature.reshape(NB, 1024, 14, 14), idx_h[3], idx_w[3])
    Wc1 = A[:, 0:7, 0:7].reshape(1024, 49).astype(np.float32)
    Wc2 = A[:, 7:14, 0:7].reshape(1024, 49).astype(np.float32)
    Wc4 = A[:, 7:14, 7:14].reshape(1024, 49).astype(np.float32)

    in_maps = []
    for m in range(NCORES):
        cs = slice(m * 128, (m + 1) * 128)
        cs3 = slice(m * 160, (m + 1) * 160)
        xin = np.concatenate([
            x1c[:, cs],
            x2c[:, cs],
            shc[:, cs],
            x3[:, cs3].reshape(NB, 128, F3),
        ], axis=2)                                   # [64, 128, 1127]
        wfull = np.concatenate([
            Wc1[cs],
            Wc2[cs],
            Wc4[cs],
            Ws3[cs3].reshape(128, F3),
        ], axis=1) * W_SCALE                         # [128, 1127]
        # First F_ROW columns stream row-major; the last C_PE columns
        # stream transposed for the PE lane.
        xr = np.ascontiguousarray(
            xin[:, :, :F_ROW].transpose(1, 0, 2), dtype=np.float16)
        xt = np.ascontiguousarray(
            xin[:, :, F_ROW:].transpose(1, 2, 0), dtype=np.float16)
        in_maps.append({'xrow': xr, 'xtr': xt,
                        'wall': np.ascontiguousarray(wfull, np.float16)})
    return in_maps


def _ensure_ntff_hook():
    """Make `trace=True` (e.g. BASS_TRACE=1) work under axon even when the
    image's antenv package lacks axon_hooks: register an equivalent module
    backed by the ctypes NTFF hook from trn_agent_boot."""
    import sys
    import types
    try:
        import antenv.axon_hooks  # noqa: F401
        return
    except Exception:
        pass
    try:
        from trn_agent_boot import trn_boot
        hook = trn_boot._ntff_profile_via_ctypes('/opt/axon/libaxon_pjrt.so')
        mod = types.ModuleType('antenv.axon_hooks')
        mod.get_axon_ntff_profile_hook = lambda: hook
        mod.set_axon_ntff_profile_hook = lambda h: None
        sys.modules['antenv.axon_hooks'] = mod
    except Exception:
        pass


def _prepare(x1, x2, x3, share_feature, c_w, conv3d_w, lin_w, lin_b,
             idx_h, idx_w):
    A, Ws3 = _build_fold(c_w, conv3d_w, lin_w, lin_b, idx_h, idx_w)
    in_maps = _shard_inputs(x1, x2, x3, share_feature, A, Ws3, lin_b,
                            idx_h, idx_w)
    nc = _build_bass()
    return in_maps, nc


def kernel(x1, x2, x3, share_feature, c_w, conv3d_w, lin_w, lin_b,
           idx_h, idx_w):
    x1, x2, x3 = np.asarray(x1), np.asarray(x2), np.asarray(x3)
    share_feature = np.asarray(share_feature)
    c_w, conv3d_w = np.asarray(c_w), np.asarray(conv3d_w)
    lin_w, lin_b = np.asarray(lin_w), np.asarray(lin_b)
    idx_h, idx_w = np.asarray(idx_h), np.asarray(idx_w)
    _ensure_ntff_hook()
    in_maps, nc = _prepare(x1, x2, x3, share_feature, c_w, conv3d_w,
                           lin_w, lin_b, idx_h, idx_w)
    res = run_bass_kernel_spmd(nc, in_maps, core_ids=list(range(NCORES)))
    parts = np.stack([np.asarray(r['out'], np.float64).sum(axis=0)
                      for r in res.results])                  # [8, 64]
    return (parts.sum(axis=0) / W_SCALE + float(lin_b[0])) \
        .astype(np.float32).reshape(NB, 1)
